# revision 21
# baseline (speedup 1.0000x reference)
"""Trainium2 Bass kernel for nn_PolicyNetwork (GRU + MLP head).

Strategy: data-parallel over batch (B=256 -> 32 per core, 8 cores).
All on-device tensors use the "transposed" layout [feature, batch] so the
512-step GRU recurrence needs no per-step transposes:
  - x_proj precompute: x_projT[g, t*b] = W_ih @ x^T  (streamed via DMA-transpose
    of bf16 x tiles), written to DRAM scratch, biases folded in.
  - recurrence: h_projT = W_hh^T-stationary matmuls streaming hT (N=32),
    gates on DVE/ACT in [128, chunk, 32] tiles, h kept as bf16 [128, 4, 32].
  - head: two small W-stationary matmuls + tanh.
Host side: weights pre-transposed + cast to bf16, biases pre-summed/reshaped,
output gathered and transposed back.
"""

import numpy as np
import ml_dtypes
from contextlib import ExitStack

import concourse.bass as bass
import concourse.bacc as bacc
import concourse.tile as tile
import concourse.mybir as mybir
from concourse.bass_utils import run_bass_kernel_spmd

T, B, D, H, M, A = 512, 256, 256, 512, 512, 64
NCORES = 8
BS = B // NCORES          # 32 batch per core
G = 3 * H                 # 1536 gate width
MCH = G // 128            # 12 gate chunks
KH = H // 128             # 4 hidden chunks
KD = D // 128             # 2 input chunks
MH = M // 128             # 4 mlp chunks
TBC = 512                 # tb-chunk size for x_proj
PREF = 16                 # steps per xp prefetch batch (PREF*BS == TBC)

f32 = mybir.dt.float32
bf16 = mybir.dt.bfloat16
AF = mybir.ActivationFunctionType
bf16_np = ml_dtypes.bfloat16


WHH_FP8 = False         # store W_hh as fp8e4m3 scaled by 16 (faster LDWEIGHTS)
FP8_SCALE = 16.0

def build(nsteps: int = T):
    nc = bacc.Bacc("TRN2", target_bir_lowering=False, debug=False)
    tbn = nsteps * BS
    wdt = mybir.dt.float8e4 if WHH_FP8 else bf16

    x_bf = nc.dram_tensor("x_bf", [tbn, D], bf16, kind="ExternalInput").ap()
    wihT = nc.dram_tensor("wihT", [D, G], bf16, kind="ExternalInput").ap()
    whhT = nc.dram_tensor("whhT", [H, G], wdt, kind="ExternalInput").ap()
    w1T = nc.dram_tensor("w1T", [H, M], bf16, kind="ExternalInput").ap()
    w2T = nc.dram_tensor("w2T", [M, A], bf16, kind="ExternalInput").ap()
    bsum = nc.dram_tensor("bsum", [128, MCH], f32, kind="ExternalInput").ap()
    b1T = nc.dram_tensor("b1T", [128, MH], f32, kind="ExternalInput").ap()
    b2c = nc.dram_tensor("b2c", [A, 1], f32, kind="ExternalInput").ap()
    outT = nc.dram_tensor("outT", [A, BS], f32, kind="ExternalOutput").ap()
    nchunks = tbn // TBC
    # Per-chunk x_projT scratch ([m-chunk, partition, tb-in-chunk]) — separate
    # DRAM tensors per chunk so the recurrence window's prefetch depends only
    # on its own chunk's writes, letting x_proj interleave with the recurrence.
    xpT = [
        nc.dram_tensor(f"xpT{c}", [MCH, 128, TBC], bf16).ap()
        for c in range(nchunks)
    ]

    with tile.TileContext(nc) as tc, ExitStack() as ctx:
        wpool = ctx.enter_context(tc.tile_pool(name="weights", bufs=1))

        wih_sb = wpool.tile([128, KD, G], bf16, tag="wih")
        for k in range(KD):
            nc.sync.dma_start(wih_sb[:, k, :], wihT[k * 128:(k + 1) * 128, :])
        whh_sb = wpool.tile([128, KH, G], wdt, tag="whh")
        for k in range(KH):
            nc.sync.dma_start(whh_sb[:, k, :], whhT[k * 128:(k + 1) * 128, :])
        w1_sb = wpool.tile([128, KH, M], bf16, tag="w1")
        for k in range(KH):
            nc.sync.dma_start(w1_sb[:, k, :], w1T[k * 128:(k + 1) * 128, :])
        w2_sb = wpool.tile([128, MH, A], bf16, tag="w2")
        for k in range(MH):
            nc.sync.dma_start(w2_sb[:, k, :], w2T[k * 128:(k + 1) * 128, :])
        bsum_sb = wpool.tile([128, MCH], f32, tag="bsum")
        nc.sync.dma_start(bsum_sb[:], bsum[:, :])
        b1_sb = wpool.tile([128, MH], f32, tag="b1")
        nc.sync.dma_start(b1_sb[:], b1T[:, :])
        b2_sb = wpool.tile([A, 1], f32, tag="b2")
        nc.sync.dma_start(b2_sb[:], b2c[:, :])

        # ---- Phase 1: x_projT = W_ih @ x^T + (b_ih + b_hh), to DRAM scratch.
        # xpT row order is permuted so half-p gate slices are contiguous:
        # rows = [r0,r1,z0,z1, r2,r3,z2,z3, n0,n1,n2,n3] (gate chunk m -> POS[m])
        POS = {0: 0, 1: 1, 4: 2, 5: 3, 2: 4, 3: 5, 6: 6, 7: 7,
               8: 8, 9: 9, 10: 10, 11: 11}
        xpool = ctx.enter_context(tc.tile_pool(name="xproj", bufs=3))
        xppsum = ctx.enter_context(
            tc.tile_pool(name="xproj_psum", bufs=2, space="PSUM"))

        xT_tiles = {}

        def xproj_load(c):
            xT = xpool.tile([128, KD, TBC], bf16, tag="xT")
            for k in range(KD):
                nc.sync.dma_start_transpose(
                    xT[:, k, :],
                    x_bf[c * TBC:(c + 1) * TBC, k * 128:(k + 1) * 128],
                )
            xT_tiles[c] = xT

        def xproj_mtile(c, m):
            xT = xT_tiles[c]
            ps = xppsum.tile([128, TBC], f32, tag="p512")
            for k in range(KD):
                nc.tensor.matmul(
                    ps[:],
                    wih_sb[:, k, m * 128:(m + 1) * 128],
                    xT[:, k, :],
                    start=(k == 0),
                    stop=(k == KD - 1),
                )
            xp = xpool.tile([128, TBC], bf16, tag="xp")
            if m % 2 == 0:
                nc.scalar.activation(
                    xp[:], ps[:], AF.Identity, bias=bsum_sb[:, m:m + 1]
                )
            else:
                nc.vector.tensor_scalar_add(xp[:], ps[:], bsum_sb[:, m:m + 1])
            nc.sync.dma_start(xpT[c][POS[m], :, :], xp[:])

        # Prologue: first two chunks computed up front; later chunks are
        # sprinkled one m-tile per recurrence step to fill PE bubbles.
        for c in range(min(2, nchunks)):
            xproj_load(c)
            for m in range(MCH):
                xproj_mtile(c, m)

        # ---- Phase 2: GRU recurrence over nsteps, half-split pipeline.
        # Half p covers H-chunks {2p, 2p+1}. Gate-chunk groups for half p:
        # rz rows 4p..4p+4 of xpT order (= r_{2p}, r_{2p+1}, z_{2p}, z_{2p+1}),
        # n rows 8+2p..8+2p+2. MM emission: [h0g k01][h1g k01][h0g k23][h1g k23]
        # so PE(t+1) can start on half-0 of h_{t+1} while gates of half 1 run.
        hpool = ctx.enter_context(tc.tile_pool(name="h", bufs=3))
        gpool = ctx.enter_context(tc.tile_pool(name="gates", bufs=2))
        xbpool = ctx.enter_context(tc.tile_pool(name="xpbuf", bufs=2))
        rpsum = ctx.enter_context(tc.tile_pool(name="rec_psum", bufs=2, space="PSUM"))
        rpsum1 = ctx.enter_context(tc.tile_pool(name="rec_psum1", bufs=1, space="PSUM"))

        # gate-chunk m (natural order) for half p: rz groups and n groups
        RZ_M = [[0, 1, 4, 5], [2, 3, 6, 7]]   # natural m for prz[p] rows 0..3
        N_M = [[8, 9], [10, 11]]

        h_init0 = hpool.tile([128, 2, BS], bf16, tag="h0")
        h_init1 = hpool.tile([128, 2, BS], bf16, tag="h1")
        h_cur = [h_init0, h_init1]
        nc.vector.memset(h_cur[0][:], 0.0)
        nc.vector.memset(h_cur[1][:], 0.0)

        def rhs_h(k):
            return h_cur[k // 2][:, k % 2, :]

        xp_buf = None
        for t in range(nsteps):
            bi = t % PREF
            w = t // PREF
            spr_c = w + 2            # chunk whose m-tiles ride in this window
            if bi == 0:
                if spr_c < nchunks:
                    xproj_load(spr_c)
                xp_buf = xbpool.tile([128, MCH, TBC], bf16, tag="xpbuf")
                for m in range(MCH):
                    nc.sync.dma_start(xp_buf[:, m, :], xpT[w][m, :, :])
            xs = slice(bi * BS, (bi + 1) * BS)

            prz0 = rpsum.tile([128, 4, BS], f32, tag="prz0")
            prz1 = rpsum.tile([128, 4, BS], f32, tag="prz1")
            pn0 = rpsum1.tile([128, 2, BS], f32, tag="pn0")
            pn1 = rpsum1.tile([128, 2, BS], f32, tag="pn1")
            prz = [prz0, prz1]
            pn = [pn0, pn1]

            # One accumulation group per PSUM bank (start=True zeroes the whole
            # 2KB bank): start on the first MM into the tile, stop on the last.
            def mm_batch(p, ks):
                first = ks[0] == 0
                last = ks[-1] == KH - 1
                for i, m in enumerate(RZ_M[p]):
                    for k in ks:
                        nc.tensor.matmul(
                            prz[p][:, i, :],
                            whh_sb[:, k, m * 128:(m + 1) * 128],
                            rhs_h(k),
                            start=(first and i == 0 and k == ks[0]),
                            stop=(last and i == len(RZ_M[p]) - 1 and k == ks[-1]),
                        )
                for i, m in enumerate(N_M[p]):
                    for k in ks:
                        nc.tensor.matmul(
                            pn[p][:, i, :],
                            whh_sb[:, k, m * 128:(m + 1) * 128],
                            rhs_h(k),
                            start=(first and i == 0 and k == ks[0]),
                            stop=(last and i == len(N_M[p]) - 1 and k == ks[-1]),
                        )

            mm_batch(0, (0, 1))
            mm_batch(1, (0, 1))
            mm_batch(0, (2, 3))
            mm_batch(1, (2, 3))
            # Sprinkled x_proj m-tile: independent PE work queued behind this
            # step's matmuls, fills the PE idle window while gates compute.
            if spr_c < nchunks and bi < MCH:
                xproj_mtile(spr_c, bi)

            h_new = [None, None]
            inv = 1.0 / FP8_SCALE if WHH_FP8 else 1.0
            for p in range(2):
                rz_pre = gpool.tile([128, 4, BS], bf16, tag=f"rzp{p}")
                if WHH_FP8:
                    nc.vector.scalar_tensor_tensor(
                        rz_pre[:], prz[p][:], inv, xp_buf[:, 4 * p:4 * p + 4, xs],
                        op0=mybir.AluOpType.mult, op1=mybir.AluOpType.add)
                else:
                    nc.vector.tensor_add(rz_pre[:], prz[p][:], xp_buf[:, 4 * p:4 * p + 4, xs])
                rz = gpool.tile([128, 4, BS], bf16, tag=f"rz{p}")
                nc.scalar.activation(rz[:], rz_pre[:], AF.Sigmoid)
                t1 = gpool.tile([128, 2, BS], bf16, tag=f"t1{p}")
                if WHH_FP8:
                    nc.vector.scalar_tensor_tensor(
                        t1[:], pn[p][:], inv, rz[:, 0:2, :],
                        op0=mybir.AluOpType.mult, op1=mybir.AluOpType.mult)
                else:
                    nc.vector.tensor_mul(t1[:], pn[p][:], rz[:, 0:2, :])
                t2 = gpool.tile([128, 2, BS], bf16, tag=f"t2{p}")
                nc.vector.tensor_add(t2[:], t1[:], xp_buf[:, 8 + 2 * p:8 + 2 * p + 2, xs])
                n_g = gpool.tile([128, 2, BS], bf16, tag=f"n{p}")
                nc.scalar.activation(n_g[:], t2[:], AF.Tanh)
                d_g = gpool.tile([128, 2, BS], bf16, tag=f"d{p}")
                nc.vector.tensor_sub(d_g[:], h_cur[p][:], n_g[:])
                u_g = gpool.tile([128, 2, BS], bf16, tag=f"u{p}")
                nc.vector.tensor_mul(u_g[:], d_g[:], rz[:, 2:4, :])
                hn = hpool.tile([128, 2, BS], bf16, tag=f"h{p}")
                nc.vector.tensor_add(hn[:], u_g[:], n_g[:])
                h_new[p] = hn
            h_cur = h_new

        # ---- Phase 3: MLP head
        ps_hid = rpsum.tile([128, MH, BS], f32, tag="prz0")
        for mh in range(MH):
            for k in range(KH):
                nc.tensor.matmul(
                    ps_hid[:, mh, :],
                    w1_sb[:, k, mh * 128:(mh + 1) * 128],
                    rhs_h(k),
                    start=(mh == 0 and k == 0),
                    stop=(mh == MH - 1 and k == KH - 1),
                )
        hid = gpool.tile([128, MH, BS], bf16, tag="hid")
        for mh in range(MH):
            nc.scalar.activation(
                hid[:, mh, :], ps_hid[:, mh, :], AF.Tanh, bias=b1_sb[:, mh:mh + 1]
            )
        ps_act = rpsum1.tile([A, BS], f32, tag="pn0")
        for k in range(MH):
            nc.tensor.matmul(
                ps_act[:],
                w2_sb[:, k, :],
                hid[:, k, :],
                start=(k == 0),
                stop=(k == MH - 1),
            )
        act = gpool.tile([A, BS], f32, tag="act")
        nc.scalar.activation(act[:], ps_act[:], AF.Tanh, bias=b2_sb[:, 0:1])
        nc.sync.dma_start(outT[:, :], act[:])

    nc.compile()
    return nc


def prep_inputs(x, W_ih, W_hh, b_ih, b_hh, W1, b1, W2, b2, nsteps: int = T):
    """Host-side prep: transpose/cast weights, shard x over batch."""
    x = np.asarray(x, dtype=np.float32)[:nsteps]
    whh_t = np.ascontiguousarray(np.asarray(W_hh, np.float32).T)
    if WHH_FP8:
        whh_in = (whh_t * FP8_SCALE).astype(ml_dtypes.float8_e4m3)
    else:
        whh_in = whh_t.astype(bf16_np)
    common = {
        "wihT": np.ascontiguousarray(np.asarray(W_ih).T).astype(bf16_np),
        "whhT": whh_in,
        "w1T": np.ascontiguousarray(np.asarray(W1).T).astype(bf16_np),
        "w2T": np.ascontiguousarray(np.asarray(W2).T).astype(bf16_np),
        "bsum": np.ascontiguousarray(
            (np.asarray(b_ih, np.float32) + np.asarray(b_hh, np.float32))
            .reshape(MCH, 128).T),
        "b1T": np.ascontiguousarray(np.asarray(b1, np.float32).reshape(MH, 128).T),
        "b2c": np.ascontiguousarray(np.asarray(b2, np.float32).reshape(A, 1)),
    }
    in_maps = []
    for i in range(NCORES):
        shard = x[:, i * BS:(i + 1) * BS, :].reshape(nsteps * BS, D)
        m = dict(common)
        m["x_bf"] = np.ascontiguousarray(shard).astype(bf16_np)
        in_maps.append(m)
    return in_maps


_CACHE = {}


def run(inputs: dict, nsteps: int = T, trace: bool = False):
    key = nsteps
    if key not in _CACHE:
        _CACHE[key] = build(nsteps)
    nc = _CACHE[key]
    in_maps = prep_inputs(**inputs, nsteps=nsteps)
    res = run_bass_kernel_spmd(
        nc, in_maps, core_ids=list(range(NCORES)), trace=trace
    )
    outs = [r["outT"] for r in res.results]
    full = np.concatenate([o.T for o in outs], axis=0)
    return full.astype(np.float32), res


def kernel(**inputs) -> np.ndarray:
    out, _ = run(inputs)
    return out
